# revision 1
# baseline (speedup 1.0000x reference)
"""Spatial-reduction attention (PVT-style) on 8 TRN2 NeuronCores.

Strategy: pure data-parallel over batch (B=8 -> 1 batch element per core,
zero collectives). Per core, everything is computed in "feature-major"
(transposed) layout so that the attention-weight matrix E^T = exp(S^T)
lands with the context dim m on partitions -- exactly what the PV matmul
needs as its moving operand, so the big attention tensor is never
transposed on chip.

Key tricks:
  - conv(stride 2, 2x2) == patch-merge matmul; patches are gathered
    host-side, only for the m positions with mask!=0 (mask compression,
    1024 -> M_pad ~ 640), since masked context positions contribute
    nothing to the attention output.
  - mask + softmax denominator are folded into the PV matmul: the
    stationary operand V'' has 65 columns per head (64 = mask*V, 1 = mask),
    so row 64 of the PV output is the softmax denominator.
  - layernorm's ln_w/ln_b are folded into Wkv host-side; on-chip LN is a
    pure standardize using ones-matmul column stats + partition broadcast.
  - all matmuls run in bf16 (full PE rate).
  - scores PSUM is one 5-bank tile; exp per head is split [4 banks]+[1
    bank] so the next head's first 4 score matmuls only wait on the big
    exp read and the 5th decouples -> ACT (exp) paces the steady state at
    ~2.7us per head-slot with PE trailing just under it.
  - per head-slot the PE runs: scores mc0-3 (half-array paired), the
    previous head's PV chain, scores mc4, and one gap-filler chain --
    Q-proj for the next chunk (slots 0-3, per-head-pair so only the pair
    needed first blocks) or out-proj for the previous chunk (slots 4-7,
    after all divides landed).
  - prologue: inputs arrive as a few large DMAs (SWDGE issue is ~0.7us
    each); Q(0) fills the PE during the DMA wait (also HAM-warms it);
    Q(1) fills the LN-chain latency; a dummy exp preloads the ACT exp
    table before the steady loop.
"""

import math
import numpy as np

N_SEQ = 4096
DIM = 512
HEADS = 8
DH = 64
INNER = HEADS * DH
SR = 2
SCALE = DH ** -0.5
LN_EPS = 1e-5
B = 8
NCHUNK = 512          # n-tile size of the main loop
EH = DH + 1           # 65: V'' columns per head (64 V + 1 mask/denominator)


def _ensure_path():
    try:
        import concourse.bass  # noqa: F401
    except ImportError:
        import sys
        for p in ("/opt/trn_rl_repo", "/root/.axon_site/_ro/trn_rl_repo"):
            if p not in sys.path:
                sys.path.append(p)


def _m_pieces(m_pad):
    """Split [0, m_pad) into 128-aligned pieces of at most 512, so each
    piece covers whole m-chunks."""
    if m_pad <= 512:
        return [(0, m_pad)]
    nmc = m_pad // 128
    a = min(3, nmc - 1)
    return [(0, a * 128), (a * 128, m_pad)]


def _build(m_pad):
    _ensure_path()
    import concourse.bass as bass  # noqa: F401
    import concourse.mybir as mybir
    import concourse.tile as tile
    from concourse import bacc

    f32 = mybir.dt.float32
    bf16 = mybir.dt.bfloat16
    FT = mybir.ActivationFunctionType
    OP = mybir.AluOpType

    nmc = m_pad // 128
    assert nmc == 5, f"optimized build assumes nmc=5, got {nmc}"
    nA = nmc - 1           # m-chunks covered by the big exp instruction
    pieces = _m_pieces(m_pad)
    n_nc = N_SEQ // NCHUNK

    nc = bacc.Bacc()

    xt_e = nc.declare_dram_parameter("xt", [DIM, N_SEQ], bf16, isOutput=False)
    xp_e = nc.declare_dram_parameter("xp", [4 * DIM, m_pad], bf16, isOutput=False)
    w2_e = nc.declare_dram_parameter("w2", [4 * DIM, DIM], bf16, isOutput=False)
    wq_e = nc.declare_dram_parameter("wq", [128, 4, DIM], bf16, isOutput=False)
    wk_e = nc.declare_dram_parameter("wk", [128, 4, DIM], bf16, isOutput=False)
    wv_e = nc.declare_dram_parameter("wv", [128, 4, DIM], bf16, isOutput=False)
    wp_e = nc.declare_dram_parameter("wp", [128, 4, DIM], bf16, isOutput=False)
    convb_e = nc.declare_dram_parameter("convb", [128, 4], f32, isOutput=False)
    bk_e = nc.declare_dram_parameter("bk", [128, 4], f32, isOutput=False)
    bv_e = nc.declare_dram_parameter("bv", [DIM], f32, isOutput=False)
    bp_e = nc.declare_dram_parameter("bp", [128, 4], f32, isOutput=False)
    maskc_e = nc.declare_dram_parameter("maskc", [128, nmc], f32, isOutput=False)
    out_e = nc.declare_dram_parameter("out", [DIM, N_SEQ], f32, isOutput=True)

    def r(ap):
        return ap

    from contextlib import ExitStack

    with tile.TileContext(nc) as tc:
        with ExitStack() as stk:
            def pool(name, bufs, space="SBUF"):
                return stk.enter_context(
                    tc.tile_pool(name=name, bufs=bufs, space=space))

            wpool = pool("wts", 1)
            cpool = pool("consts", 1)
            xpcp = pool("stream", 1)
            w2p = pool("w2s", 1)
            ctxp = pool("ctx", 1)
            sqp = pool("sqs", 2)
            kvp = pool("kv", 1)
            xtp = pool("xtq", 3)
            qp = pool("qq", 2)
            ep = pool("ee", 2)
            opool = pool("oo", 2)
            yp = pool("yy", 3)
            smp = pool("small", 1)
            r1p = pool("r1p", 2)
            bcp = pool("bc", 2)
            rbp = pool("rbp", 2)
            s5pool = pool("ps_s5", 1, space="PSUM")   # 5 banks: scores/patch
            pvp = pool("ps_pv", 1, space="PSUM")      # 1 bank: PV out
            qpp = pool("ps_q", 1, space="PSUM")       # 1 bank: Q/K chains
            fpp = pool("ps_f", 1, space="PSUM")       # 1 bank: out/V chains

            # ---- constants; dummy exp at t=0 preloads the ACT exp table ----
            eps_sb = cpool.tile([1, 1], f32, tag="eps")
            nc.vector.memset(eps_sb[:], LN_EPS)
            warm_sb = cpool.tile([1, 1], f32, tag="warm")
            nc.scalar.activation(out=warm_sb[:], in_=eps_sb[:], func=FT.Exp)
            warmb_sb = cpool.tile([2, 1], f32, tag="warmb")
            nc.gpsimd.partition_broadcast(out_ap=warmb_sb[:], in_ap=eps_sb[:])

            wq_sb = wpool.tile([128, 4, DIM], bf16, tag="wq")
            nc.gpsimd.dma_start(out=wq_sb[:], in_=wq_e.ap())
            ones_sb = cpool.tile([128, 1], bf16, tag="ones")
            nc.vector.memset(ones_sb[:], 1.0)
            ones8_sb = cpool.tile([128, 8], f32, tag="ones8")
            nc.vector.memset(ones8_sb[:], 1.0)

            # ---- streaming input DMAs: few large transfers, piece-0 first ----
            xt_r = xt_e.ap().rearrange("(cc p) n -> p cc n", p=128)  # [128,4,N]
            xt_tiles = {}
            for k in (0, 1):
                xt = xtp.tile([128, 4, NCHUNK], bf16, tag="xt")
                nc.sync.dma_start(
                    out=xt[:], in_=xt_r[:, :, k * NCHUNK:(k + 1) * NCHUNK])
                xt_tiles[k] = xt

            xp_r = xp_e.ap().rearrange("(kc p) m -> kc p m", p=128)   # [16,128,m]
            w2_r = w2_e.ap().rearrange("(g kc p) co -> g p kc co", g=2, p=128)
            xp_t = []
            for kc in range(16):
                xpk = xpcp.tile([128, m_pad], bf16, tag=f"xp{kc}")
                xp_t.append(xpk)
            w2_t0 = w2p.tile([128, 8, DIM], bf16, tag="w20")
            w2_t1 = w2p.tile([128, 8, DIM], bf16, tag="w21")
            w2_t = [w2_t0, w2_t1]
            # big streams all on the sync queue, in patch consumption order
            nc.sync.dma_start(out=w2_t[0][:], in_=w2_r[0])
            for kc in range(8):
                nc.sync.dma_start(out=xp_t[kc][:], in_=xp_r[kc, :, :])
            nc.sync.dma_start(out=w2_t[1][:], in_=w2_r[1])
            for kc in range(8, 16):
                nc.sync.dma_start(out=xp_t[kc][:], in_=xp_r[kc, :, :])
            # remaining weights/consts (needed later in the prologue)
            wk_sb = wpool.tile([128, 4, DIM], bf16, tag="wk")
            nc.gpsimd.dma_start(out=wk_sb[:], in_=wk_e.ap())
            wv_sb = wpool.tile([128, 4, DIM], bf16, tag="wv")
            nc.gpsimd.dma_start(out=wv_sb[:], in_=wv_e.ap())
            wp_sb = wpool.tile([128, 4, DIM], bf16, tag="wp")
            nc.gpsimd.dma_start(out=wp_sb[:], in_=wp_e.ap())
            convb_sb = cpool.tile([128, 4], f32, tag="convb")
            nc.gpsimd.dma_start(out=convb_sb[:], in_=convb_e.ap())
            bk_sb = cpool.tile([128, 4], f32, tag="bk")
            nc.gpsimd.dma_start(out=bk_sb[:], in_=bk_e.ap())
            bp_sb = cpool.tile([128, 4], f32, tag="bp")
            nc.gpsimd.dma_start(out=bp_sb[:], in_=bp_e.ap())
            maskc_sb = cpool.tile([128, nmc], f32, tag="maskc")
            nc.gpsimd.dma_start(out=maskc_sb[:], in_=maskc_e.ap())
            bv_bc = cpool.tile([128, DIM], f32, tag="bvbc")
            nc.gpsimd.dma_start(out=bv_bc[:], in_=bv_e.ap().partition_broadcast(128))
            xt2 = xtp.tile([128, 4, NCHUNK], bf16, tag="xt")
            nc.sync.dma_start(out=xt2[:], in_=xt_r[:, :, 2 * NCHUNK:3 * NCHUNK])
            xt_tiles[2] = xt2

            def w2kc(kc):
                return w2_t[kc // 8][:, kc % 8, :]

            def xpkc(kc, p0, p1):
                return xp_t[kc][:, p0:p1]

            # ---- persistent PSUM scores tiles: 4-bank A + 1-bank D ----
            # (separate tiles so Tile's tile-granular dep tracking lets the
            # next head's mc4 score matmul bypass the big eA read)
            sA4 = s5pool.tile([128, nA, NCHUNK], f32, tag="sA4")
            sDt = s5pool.tile([128, NCHUNK], f32, tag="sDt")

            # ---- Q projection helpers ----
            def q_chain(xt_sb, q_sb, ic, psum_pool=None, psum_tag="q"):
                pool_ = psum_pool or qpp
                ps = pool_.tile([128, NCHUNK], f32, tag=psum_tag)
                for cc in range(4):
                    nc.tensor.matmul(
                        ps[:],
                        lhsT=r(wq_sb[:, cc, ic * 128:(ic + 1) * 128]),
                        rhs=r(xt_sb[:, cc, :]),
                        start=(cc == 0), stop=(cc == 3),
                    )
                nc.vector.tensor_copy(out=q_sb[:, ic, :], in_=ps[:])

            def q_swap_dma(q_sb, q_sw):
                # partition-shift copies via DMA (slow path on DVE)
                nc.gpsimd.dma_start(out=q_sw[0:64, :, :], in_=q_sb[64:128, :, :])
                nc.gpsimd.dma_start(out=q_sw[64:128, :, :], in_=q_sb[0:64, :, :])

            # ---- HAM warm-up: ~3.5us of tiny dependency-free matmuls so
            # Q(0) and the patch run at 2.4 GHz from the start ----
            warm_ps = pvp.tile([8, 64], f32, tag="pv")
            for _ in range(52):
                nc.tensor.matmul(warm_ps[0:8, 0:8], lhsT=r(ones8_sb[0:1, :]),
                                 rhs=r(ones8_sb[0:1, :]),
                                 start=True, stop=True)

            # ---- prologue Q for chunk 0 (PE work during DMA wait);
            # alternate PSUM banks so the q-copy WAR doesn't serialize ----
            q_tiles = {}
            q_sb0 = qp.tile([128, 4, NCHUNK], bf16, tag="q")
            q_sw0 = qp.tile([128, 4, NCHUNK], bf16, tag="qsw")
            q_tiles[0] = (q_sb0, q_sw0)
            for ic in range(4):
                q_chain(xt_tiles[0], q_tiles[0][0], ic,
                        psum_pool=(qpp if ic % 2 == 0 else fpp),
                        psum_tag=("q" if ic % 2 == 0 else "fin"))
            q_swap_dma(*q_tiles[0])

            # ---- phase 1: patch-merge ctx^T (feature-major) + layernorm ----
            ctx_raw = ctxp.tile([128, 4, m_pad], bf16, tag="craw")
            ctxn = ctxp.tile([128, 4, m_pad], bf16, tag="cn")
            k_sb = kvp.tile([128, 4, m_pad], bf16, tag="k")
            k_sw = kvp.tile([128, 4, m_pad], bf16, tag="ksw")
            v2_sb = kvp.tile([128, nmc, HEADS * EH], bf16, tag="v2")
            bv3 = bv_bc[:].rearrange("p (h d) -> p h d", d=DH)

            def patch_stats_ln(p0, p1):
                """Patch-merge matmuls + bias + column stats + LN scalar
                chain for one piece. PE part: 64 patch MMs + 8 stats MMs."""
                pw = p1 - p0
                kc_order = list(range(16))
                for ki, kc in enumerate(kc_order):
                    for cco in range(4):
                        nc.tensor.matmul(
                            sA4[:, cco, :pw],
                            lhsT=r(w2kc(kc)[:, cco * 128:(cco + 1) * 128]),
                            rhs=r(xpkc(kc, p0, p1)),
                            start=(ki == 0), stop=(ki == 15),
                        )
                for cco in range(4):
                    nc.vector.tensor_scalar(
                        out=ctx_raw[:, cco, p0:p1], in0=sA4[:, cco, :pw],
                        scalar1=convb_sb[:, cco:cco + 1], scalar2=None,
                        op0=OP.add,
                    )
                mu_ps = pvp.tile([EH, NCHUNK], f32, tag="pv")
                ss_ps = fpp.tile([128, NCHUNK], f32, tag="fin")
                for cc in range(4):
                    sq_s = sqp.tile([128, NCHUNK], bf16, tag="sqs")
                    nc.vector.tensor_tensor(
                        out=sq_s[:, :pw], in0=ctx_raw[:, cc, p0:p1],
                        in1=ctx_raw[:, cc, p0:p1], op=OP.mult,
                    )
                    nc.tensor.matmul(
                        mu_ps[0:1, :pw], lhsT=r(ones_sb[:]),
                        rhs=r(ctx_raw[:, cc, p0:p1]),
                        start=(cc == 0), stop=(cc == 3),
                    )
                    nc.tensor.matmul(
                        ss_ps[0:1, :pw], lhsT=r(ones_sb[:]),
                        rhs=r(sq_s[:, :pw]),
                        start=(cc == 0), stop=(cc == 3),
                    )
                m1n = smp.tile([1, NCHUNK], f32, tag="m1n")
                nc.vector.tensor_scalar(
                    out=m1n[:, :pw], in0=mu_ps[0:1, :pw],
                    scalar1=-1.0 / DIM, scalar2=None, op0=OP.mult,
                )
                v1 = smp.tile([1, NCHUNK], f32, tag="v1")
                nc.vector.tensor_scalar(
                    out=v1[:, :pw], in0=ss_ps[0:1, :pw],
                    scalar1=1.0 / DIM, scalar2=None, op0=OP.mult,
                )
                m2 = smp.tile([1, NCHUNK], f32, tag="m2")
                nc.vector.tensor_tensor(
                    out=m2[:, :pw], in0=m1n[:, :pw], in1=m1n[:, :pw], op=OP.mult
                )
                var = smp.tile([1, NCHUNK], f32, tag="var")
                nc.vector.tensor_tensor(
                    out=var[:, :pw], in0=v1[:, :pw], in1=m2[:, :pw],
                    op=OP.subtract,
                )
                std = smp.tile([1, NCHUNK], f32, tag="std")
                nc.scalar.activation(
                    out=std[:, :pw], in_=var[:, :pw], func=FT.Sqrt,
                    bias=eps_sb[:],
                )
                # reciprocal via partition-transpose (DVE recip on [1, pw] is
                # ~7ns/elem; on [128, pw/128] it is ~free)
                npc = pw // 128
                stdT = smp.tile([128, 4], f32, tag="stdT")
                nc.gpsimd.dma_start(
                    out=stdT[:, :npc].rearrange("p c -> p (c)"),
                    in_=std[:, :pw])
                rstdT = smp.tile([128, 4], f32, tag="rstdT")
                nc.vector.reciprocal(out=rstdT[:, :npc], in_=stdT[:, :npc])
                rstd = smp.tile([1, NCHUNK], f32, tag="rstd")
                nc.gpsimd.dma_start(
                    out=rstd[:, :pw],
                    in_=rstdT[:, :npc].rearrange("p c -> p (c)"))
                tsh = smp.tile([1, NCHUNK], f32, tag="tsh")
                nc.vector.tensor_tensor(
                    out=tsh[:, :pw], in0=m1n[:, :pw], in1=rstd[:, :pw],
                    op=OP.mult,
                )
                r_bc = bcp.tile([128, NCHUNK], f32, tag="rbc")
                nc.gpsimd.partition_broadcast(out_ap=r_bc[:, :pw],
                                              in_ap=rstd[:, :pw])
                t_bc = bcp.tile([128, NCHUNK], f32, tag="tbc")
                nc.gpsimd.partition_broadcast(out_ap=t_bc[:, :pw],
                                              in_ap=tsh[:, :pw])
                for cc in range(4):
                    nc.vector.tensor_tensor(
                        out=ctxn[:, cc, p0:p1], in0=ctx_raw[:, cc, p0:p1],
                        in1=r_bc[:, :pw], op=OP.mult,
                    )
                    nc.vector.tensor_tensor(
                        out=ctxn[:, cc, p0:p1], in0=ctxn[:, cc, p0:p1],
                        in1=t_bc[:, :pw], op=OP.add,
                    )

            def k_piece(p0, p1):
                pw = p1 - p0
                for kc in range(4):
                    ps = qpp.tile([128, NCHUNK], f32, tag="q")
                    for cc in range(4):
                        nc.tensor.matmul(
                            ps[:, :pw],
                            lhsT=r(wk_sb[:, cc, kc * 128:(kc + 1) * 128]),
                            rhs=r(ctxn[:, cc, p0:p1]),
                            start=(cc == 0), stop=(cc == 3),
                        )
                    nc.vector.tensor_scalar(
                        out=k_sb[:, kc, p0:p1], in0=ps[:, :pw],
                        scalar1=bk_sb[:, kc:kc + 1], scalar2=None, op0=OP.add,
                    )
                nc.vector.tensor_copy(
                    out=k_sw[0:64, :, p0:p1], in_=k_sb[64:128, :, p0:p1])
                nc.vector.tensor_copy(
                    out=k_sw[64:128, :, p0:p1], in_=k_sb[0:64, :, p0:p1])

            def v_piece(p0, p1):
                for mc in range(p0 // 128, p1 // 128):
                    ps = fpp.tile([128, NCHUNK], f32, tag="fin")
                    for cc in range(4):
                        nc.tensor.matmul(
                            ps[:],
                            lhsT=r(ctxn[:, cc, mc * 128:(mc + 1) * 128]),
                            rhs=r(wv_sb[:, cc, :]),
                            start=(cc == 0), stop=(cc == 3),
                        )
                    v3 = v2_sb[:, mc, :].rearrange("p (h e) -> p h e", e=EH)
                    nc.vector.tensor_tensor(
                        out=v3[:, :, 0:DH],
                        in0=ps[:].rearrange("p (h d) -> p h d", d=DH),
                        in1=bv3, op=OP.add,
                    )
                    nc.vector.tensor_scalar(
                        out=v3[:, :, 0:DH], in0=v3[:, :, 0:DH],
                        scalar1=maskc_sb[:, mc:mc + 1], scalar2=None,
                        op0=OP.mult,
                    )
                    nc.vector.tensor_scalar(
                        out=v3[:, :, DH:EH],
                        in0=ones8_sb[:].rearrange("p (h u) -> p h u", u=1),
                        scalar1=maskc_sb[:, mc:mc + 1], scalar2=None,
                        op0=OP.mult,
                    )

            (pa, pb) = pieces
            patch_stats_ln(*pa)
            patch_stats_ln(*pb)
            # re-warm the exp table set (Sqrt above switched sets) while the
            # PE chews on K/V
            warm2_sb = cpool.tile([1, 1], f32, tag="warm2")
            nc.scalar.activation(out=warm2_sb[:], in_=eps_sb[:], func=FT.Exp)
            k_piece(*pa)
            v_piece(*pa)
            k_piece(*pb)
            v_piece(*pb)

            # ---- phase 3: main n-chunk loop, head-slot pipeline ----
            def scores_mm(q_sb, q_sw, h, mc):
                hc = h // 2
                half = mc % 2
                if (h % 2) == half:
                    ksrc, qsrc = k_sb, q_sb
                else:
                    ksrc, qsrc = k_sw, q_sw
                hp = half * 64
                dst = sA4[:, mc, :] if mc < nA else sDt[:]
                nc.tensor.matmul(
                    dst,
                    lhsT=r(ksrc[hp:hp + 64, hc, mc * 128:(mc + 1) * 128]),
                    rhs=r(qsrc[hp:hp + 64, hc, :]),
                    start=True, stop=True,
                )

            def pv_drain(h, eA, eD, o_st):
                pv = pvp.tile([EH, NCHUNK], f32, tag="pv")
                for mc in range(nmc):
                    src = eA[:, mc, :] if mc < nA else eD[:]
                    nc.tensor.matmul(
                        pv[:],
                        lhsT=r(v2_sb[:, mc, h * EH:(h + 1) * EH]),
                        rhs=r(src),
                        start=(mc == 0), stop=(mc == nmc - 1),
                    )
                nc.vector.tensor_copy(out=o_st[:, h, :], in_=pv[:])

            def div_stage1(g, o_st):
                dT = r1p.tile([128, 2 * NCHUNK // 128], bf16, tag="dT")
                nc.gpsimd.dma_start(
                    out=dT[:],
                    in_=o_st[DH:EH, 2 * g:2 * g + 2, :].rearrange(
                        "p a b -> p (a b)"))
                return dT

            def div_stage2(g, dT):
                rT = r1p.tile([128, 2 * NCHUNK // 128], bf16, tag="rT")
                with nc.allow_low_precision("bf16 softmax denoms"):
                    nc.vector.reciprocal(out=rT[:], in_=dT[:])
                rfl = r1p.tile([1, 2, NCHUNK], bf16, tag="rf")
                nc.gpsimd.dma_start(
                    out=rfl[:].rearrange("p a b -> p (a b)"),
                    in_=rT[:])
                rbs = []
                for j in (0, 1):
                    rb = rbp.tile([64, NCHUNK], bf16, tag="rb")
                    nc.gpsimd.partition_broadcast(
                        out_ap=rb[:], in_ap=rfl[0:1, j, :])
                    rbs.append(rb)
                return rbs

            def div_stage3(g, rbs, o_st, o_sb):
                for j, hh in enumerate((2 * g, 2 * g + 1)):
                    nc.vector.tensor_tensor(
                        out=o_sb[(hh % 2) * 64:(hh % 2) * 64 + 64,
                                 hh // 2, :],
                        in0=o_st[0:DH, hh, :], in1=rbs[j],
                        op=OP.mult,
                    )

            def out_chain(o_sb, cc, n0):
                ps = fpp.tile([128, NCHUNK], f32, tag="fin")
                for ic in range(4):
                    nc.tensor.matmul(
                        ps[:],
                        lhsT=r(wp_sb[:, ic, cc * 128:(cc + 1) * 128]),
                        rhs=r(o_sb[:, ic, :]),
                        start=(ic == 0), stop=(ic == 3),
                    )
                y_sb = yp.tile([128, NCHUNK], f32, tag="y")
                nc.vector.tensor_scalar(
                    out=y_sb[:], in0=ps[:], scalar1=bp_sb[:, cc:cc + 1],
                    scalar2=None, op0=OP.add,
                )
                nc.sync.dma_start(
                    out=out_e.ap()[cc * 128:(cc + 1) * 128, n0:n0 + NCHUNK],
                    in_=y_sb[:],
                )

            prev = None       # (h, eA, eD, o_st, o_sb)
            prev_chunk = None  # (o_sb, n0) of previous chunk, for out-proj
            pending = []      # [(due_slot, fn)] deferred divide stages
            slot = 0

            def flush(s):
                ready = [p for p in pending if p[0] <= s]
                pending[:] = [p for p in pending if p[0] > s]
                for _, fn in ready:
                    fn()
            for ni in range(n_nc):
                n0 = ni * NCHUNK
                q_sb, q_sw = q_tiles[ni]
                o_sb = opool.tile([128, 4, NCHUNK], bf16, tag="o")
                o_st = opool.tile([EH, 8, NCHUNK], bf16, tag="ost")
                if 0 <= ni <= 6:
                    qn_sb = qp.tile([128, 4, NCHUNK], bf16, tag="q")
                    qn_sw = qp.tile([128, 4, NCHUNK], bf16, tag="qsw")
                    q_tiles[ni + 1] = (qn_sb, qn_sw)
                for h in range(HEADS):
                    flush(slot)
                    # scores mc 0..3 -> banks 0..3 (wait previous eA read)
                    for mc in range(nA):
                        scores_mm(q_sb, q_sw, h, mc)
                    eA = ep.tile([128, nA, NCHUNK], bf16, tag="eA")
                    eD = ep.tile([128, NCHUNK], bf16, tag="eD")
                    nc.scalar.activation(
                        out=eA[:], in_=sA4[:], func=FT.Exp)
                    # trailing PV for the previous head-slot
                    if prev is not None:
                        ph, peA, peD, po_st, po_sb = prev
                        pv_drain(ph, peA, peD, po_st)
                    # scores mc 4 -> bank 4 (waits previous eD read only)
                    scores_mm(q_sb, q_sw, h, nA)
                    nc.scalar.activation(
                        out=eD[:], in_=sDt[:], func=FT.Exp)
                    # gap-filler chains: Q-proj (next chunk) in slots 0-3,
                    # out-proj (previous chunk) in slots 4-7
                    if h < 4:
                        if 0 <= ni <= 6:
                            q_chain(xt_tiles[ni + 1], q_tiles[ni + 1][0], h)
                            if h == 3:
                                q_swap_dma(*q_tiles[ni + 1])
                    else:
                        if prev_chunk is not None:
                            out_chain(prev_chunk[0], h - 4, prev_chunk[1])
                        if h == 6 and 1 <= ni <= 5:
                            xt = xtp.tile([128, 4, NCHUNK], bf16, tag="xt")
                            nc.sync.dma_start(
                                out=xt[:],
                                in_=xt_r[:, :,
                                         (ni + 2) * NCHUNK:(ni + 3) * NCHUNK])
                            xt_tiles[ni + 2] = xt
                    # divides trail everything (gpsimd/DVE tail work),
                    # pipelined over the next slots
                    if prev is not None and prev[0] % 2 == 1:
                        g, d_ost, d_osb = prev[0] // 2, prev[3], prev[4]
                        dT = div_stage1(g, d_ost)

                        def mk2(g=g, dT=dT, d_ost=d_ost, d_osb=d_osb, s=slot):
                            rbs = div_stage2(g, dT)
                            pending.append(
                                (s + 3,
                                 lambda: div_stage3(g, rbs, d_ost, d_osb)))
                        pending.append((slot + 2, mk2))
                    prev = (h, eA, eD, o_st, o_sb)
                    slot += 1
                prev_chunk = (o_sb, n0)

            # ---- tail: last head's PV, divide, and final out-proj ----
            ph, peA, peD, po_st, po_sb = prev
            pv_drain(ph, peA, peD, po_st)
            while pending:
                flush(slot + 10)
                slot += 10
            dT = div_stage1(3, po_st)
            rbs = div_stage2(3, dT)
            div_stage3(3, rbs, po_st, po_sb)
            for cc in range(4):
                out_chain(prev_chunk[0], cc, prev_chunk[1])

    nc.finalize()
    return nc


def _prep_inputs(x, mask, Wq, Wkv, conv_w, conv_b, ln_w, ln_b, Wp, bp, W):
    """Host-side sharding + layout prep. Returns (in_maps, m_pad)."""
    import ml_dtypes
    bf16 = ml_dtypes.bfloat16
    x = np.ascontiguousarray(np.asarray(x, dtype=np.float32))
    mask = np.asarray(mask, dtype=np.float32)
    Wq = np.asarray(Wq, dtype=np.float32)
    Wkv = np.asarray(Wkv, dtype=np.float32)
    conv_w = np.asarray(conv_w, dtype=np.float32)
    conv_b = np.asarray(conv_b, dtype=np.float32)
    ln_w = np.asarray(ln_w, dtype=np.float32)
    ln_b = np.asarray(ln_b, dtype=np.float32)
    Wp = np.asarray(Wp, dtype=np.float32)
    bp = np.asarray(bp, dtype=np.float32)

    Wm = W // SR
    kb = [int((mask[b] != 0).sum()) for b in range(B)]
    m_pad = max(256, ((max(kb) + 127) // 128) * 128)

    def rearr_w(w):  # [512, 512] -> [128, 4, 512] with [p, cc, :] = w[cc*128+p]
        return np.ascontiguousarray(w.reshape(4, 128, -1).transpose(1, 0, 2))

    def rearr_b(v):  # [512] -> [128, 4]
        return np.ascontiguousarray(v.reshape(4, 128).T)

    w2 = np.ascontiguousarray(
        conv_w.transpose(2, 3, 1, 0).reshape(4 * DIM, DIM)).astype(bf16)
    wq_in = rearr_w(Wq.T * np.float32(SCALE)).astype(bf16)
    wk_in = rearr_w((Wkv[:INNER] * ln_w).T).astype(bf16)
    wv_in = rearr_w((Wkv[INNER:] * ln_w).T).astype(bf16)
    wp_in = rearr_w(Wp.T).astype(bf16)
    bk_in = rearr_b(Wkv[:INNER] @ ln_b)
    bv_in = np.ascontiguousarray(Wkv[INNER:] @ ln_b)
    convb_in = rearr_b(conv_b)
    bp_in = rearr_b(bp)

    in_maps = []
    for b in range(B):
        xb = x[b]
        sel = np.nonzero(mask[b] != 0)[0]
        sel_pad = np.zeros(m_pad, dtype=np.int64)
        sel_pad[: len(sel)] = sel
        i = sel_pad // Wm
        j = sel_pad % Wm
        n_idx = np.stack(
            [(2 * i + di) * W + (2 * j + dj) for di in (0, 1) for dj in (0, 1)]
        )  # [4, m_pad], p = di*2+dj
        xp = xb[n_idx]  # [4, m_pad, 512]
        xp = np.ascontiguousarray(
            xp.transpose(0, 2, 1).reshape(4 * DIM, m_pad))
        maskc = (np.arange(m_pad) < len(sel)).astype(np.float32)
        maskc_in = np.ascontiguousarray(maskc.reshape(-1, 128).T)
        in_maps.append({
            "xt": np.ascontiguousarray(xb.T).astype(bf16),
            "xp": xp.astype(bf16),
            "w2": w2,
            "wq": wq_in,
            "wk": wk_in,
            "wv": wv_in,
            "wp": wp_in,
            "convb": convb_in,
            "bk": bk_in,
            "bv": bv_in,
            "bp": bp_in,
            "maskc": maskc_in,
        })
    return in_maps, m_pad


_BUILD_CACHE = {}


def kernel(x, H, W, mask, Wq, Wkv, conv_w, conv_b, ln_w, ln_b, Wp, bp,
           _results_hook=None):
    H = int(H)
    W = int(W)
    assert (H, W) == (64, 64) and x.shape == (B, N_SEQ, DIM), (H, W, x.shape)

    in_maps, m_pad = _prep_inputs(
        x, mask, Wq, Wkv, conv_w, conv_b, ln_w, ln_b, Wp, bp, W)

    if m_pad not in _BUILD_CACHE:
        _BUILD_CACHE[m_pad] = _build(m_pad)
    nc = _BUILD_CACHE[m_pad]

    _ensure_path()
    from concourse.bass_utils import run_bass_kernel_spmd

    res = run_bass_kernel_spmd(nc, in_maps, core_ids=list(range(B)))
    if _results_hook is not None:
        _results_hook(res)

    out = np.empty((B, N_SEQ, DIM), dtype=np.float32)
    for b in range(B):
        out[b] = res.results[b]["out"].T
    return out



# revision 14
# speedup vs baseline: 1.0892x; 1.0892x over previous
"""Spatial-reduction attention (PVT-style) on 8 TRN2 NeuronCores.

Strategy: pure data-parallel over batch (B=8 -> 1 batch element per core,
zero collectives). Per core, everything is computed in "feature-major"
(transposed) layout so that the attention-weight matrix E^T = exp(S^T)
lands with the context dim m on partitions -- exactly what the PV matmul
needs as its moving operand, so the big attention tensor is never
transposed on chip.

Key tricks:
  - conv(stride 2, 2x2) == patch-merge matmul; patches are gathered
    host-side, only for the m positions with mask!=0 (mask compression,
    1024 -> M_pad ~ 640), since masked context positions contribute
    nothing to the attention output.
  - mask + softmax denominator are folded into the PV matmul: the
    stationary operand V'' has 65 columns per head (64 = mask*V, 1 = mask),
    so row 64 of the PV output is the softmax denominator.
  - layernorm's ln_w/ln_b are folded into Wkv host-side; on-chip LN is a
    pure standardize using ones-matmul column stats; the per-column
    rstd/shift are produced via tiny PE transposes ([128,1]-form scalar
    chain, no DMA round-trips) and broadcast with partition-broadcast
    DMAs on the idle sync queue.
  - all matmuls run in bf16 (full PE rate).
  - inputs stream in as a handful of large contiguous [128, N] DMAs from
    a host-packed bundle (descriptor generation is ~per-row, so packing
    cuts issue cost from ~20us to ~2us); slices are ordered so the
    patch matmuls start as soon as w2+xp land.
  - scores PSUM is one 5-bank tile; exp per head is split [4 banks]+[1
    bank] so the next head's first 4 score matmuls only wait on the big
    exp read and the 5th decouples -> ACT (exp) paces the steady state at
    ~2.7us per head-slot with PE trailing just under it.
  - per head-slot the PE runs: scores mc0-3 (half-array paired), the
    previous head's PV chain, scores mc4, and one gap-filler chain --
    Q-proj for the next chunk (slots 0-3) or out-proj for the previous
    chunk (slots 4-7, after all divides landed).  Divide chains are
    scheduled 1 slot earlier than strictly needed so the slot-4
    out-proj never stalls on the last pair's divide.
  - tail: the final chunk's out-proj accumulates per-ic into the freed
    score PSUM banks while the last divide chain completes, instead of
    serializing divide -> full out-proj.
"""

import math
import numpy as np

N_SEQ = 4096
DIM = 512
HEADS = 8
DH = 64
INNER = HEADS * DH
SR = 2
SCALE = DH ** -0.5
LN_EPS = 1e-5
B = 8
NCHUNK = 512          # n-tile size of the main loop
EH = DH + 1           # 65: V'' columns per head (64 V + 1 mask/denominator)

# bundle column offsets (bf16, [128, C] host-packed)
C_WQ = 0
C_W2 = C_WQ + 4 * DIM           # 2048
C_XP = C_W2 + 16 * DIM          # 10240
C_WK = None                     # filled at build time (depends on m_pad)


def _ensure_path():
    try:
        import concourse.bass  # noqa: F401
    except ImportError:
        import sys
        for p in ("/opt/trn_rl_repo", "/root/.axon_site/_ro/trn_rl_repo"):
            if p not in sys.path:
                sys.path.append(p)


def _m_pieces(m_pad):
    """Split [0, m_pad) into 128-aligned pieces of at most 512, so each
    piece covers whole m-chunks."""
    if m_pad <= 512:
        return [(0, m_pad)]
    nmc = m_pad // 128
    a = min(3, nmc - 1)
    return [(0, a * 128), (a * 128, m_pad)]


def _bundle_cols(m_pad):
    c_wq = 0
    c_w2 = c_wq + 4 * DIM
    c_xp = c_w2 + 16 * DIM
    c_wk = c_xp + 16 * m_pad
    c_wv = c_wk + 4 * DIM
    c_wp = c_wv + 4 * DIM
    c_end = c_wp + 4 * DIM
    return dict(wq=c_wq, w2=c_w2, xp=c_xp, wk=c_wk, wv=c_wv, wp=c_wp,
                end=c_end)


def _build(m_pad):
    _ensure_path()
    import concourse.bass as bass  # noqa: F401
    import concourse.mybir as mybir
    import concourse.tile as tile
    from concourse import bacc

    f32 = mybir.dt.float32
    bf16 = mybir.dt.bfloat16
    FT = mybir.ActivationFunctionType
    OP = mybir.AluOpType

    nmc = m_pad // 128
    assert nmc == 5, f"optimized build assumes nmc=5, got {nmc}"
    nA = nmc - 1           # m-chunks covered by the big exp instruction
    pieces = _m_pieces(m_pad)
    n_nc = N_SEQ // NCHUNK
    CO = _bundle_cols(m_pad)

    nc = bacc.Bacc()

    bund_e = nc.declare_dram_parameter("bund", [128, CO["end"]], bf16,
                                       isOutput=False)
    xtb_e = nc.declare_dram_parameter("xtb", [128, n_nc, 4, NCHUNK], bf16,
                                      isOutput=False)
    cst_e = nc.declare_dram_parameter("cst", [128, 145], f32, isOutput=False)
    bv_e = nc.declare_dram_parameter("bv", [DIM], f32, isOutput=False)
    out_e = nc.declare_dram_parameter("out", [DIM, N_SEQ], f32, isOutput=True)

    def r(ap):
        return ap

    from contextlib import ExitStack

    with tile.TileContext(nc) as tc:
        with ExitStack() as stk:
            def pool(name, bufs, space="SBUF"):
                return stk.enter_context(
                    tc.tile_pool(name=name, bufs=bufs, space=space))

            bnd = pool("bnd", 1)       # bundle slices (weights, xp)
            cpool = pool("consts", 1)
            ctxp = pool("ctx", 1)
            sqp = pool("sqs", 2)
            kvp = pool("kv", 1)
            xtp = pool("xtq", 3)
            qp = pool("qq", 2)
            ep = pool("ee", 2)
            opool = pool("oo", 2)
            yp = pool("yy", 3)
            smp = pool("small", 1)
            r1p = pool("r1p", 2)
            bcp = pool("bc", 2)
            rbp = pool("rbp", 2)
            s5pool = pool("ps_s5", 1, space="PSUM")   # 5 banks: scores/patch
            pvp = pool("ps_pv", 1, space="PSUM")      # 1 bank: PV out
            qpp = pool("ps_q", 1, space="PSUM")       # 1 bank: Q/K chains
            fpp = pool("ps_f", 1, space="PSUM")       # 1 bank: out/V chains

            # ---- constants; dummy exp at t=0 preloads the ACT exp table ----
            eps_sb = cpool.tile([1, 1], f32, tag="eps")
            nc.vector.memset(eps_sb[:], LN_EPS)
            eps128 = cpool.tile([128, 1], f32, tag="eps128")
            nc.vector.memset(eps128[:], LN_EPS)
            warm_sb = cpool.tile([1, 1], f32, tag="warm")
            nc.scalar.activation(out=warm_sb[:], in_=eps_sb[:], func=FT.Exp)
            warmb_sb = cpool.tile([2, 1], f32, tag="warmb")
            nc.gpsimd.partition_broadcast(out_ap=warmb_sb[:], in_ap=eps_sb[:])

            ones_sb = cpool.tile([128, 1], bf16, tag="ones")
            nc.vector.memset(ones_sb[:], 1.0)
            ones8_sb = cpool.tile([128, 8], f32, tag="ones8")
            nc.vector.memset(ones8_sb[:], 1.0)

            # ---- streaming input DMAs: few large contiguous transfers on
            # the sync queue, in consumption order ----
            def bslice(name, c0, c1):
                t = bnd.tile([128, 4, DIM], bf16, tag=f"b_{name}")
                nc.sync.dma_start(
                    out=t[:].rearrange("p a b -> p (a b)"),
                    in_=bund_e.ap()[:, c0:c1])
                return t

            # small fp32 consts on the gpsimd queue (idle in prologue)
            cst_sb = cpool.tile([128, 145], f32, tag="cst")
            nc.gpsimd.dma_start(out=cst_sb[:], in_=cst_e.ap())
            convb_sb = cst_sb[:, 0:4]
            bk_sb = cst_sb[:, 4:8]
            bp_sb = cst_sb[:, 8:12]
            maskc_sb = cst_sb[:, 12:17]
            id128 = cst_sb[:, 17:145]
            id2 = cst_sb[0:2, 17:19]
            bv_bc = cpool.tile([128, DIM], f32, tag="bvbc")
            nc.gpsimd.dma_start(out=bv_bc[:], in_=bv_e.ap().partition_broadcast(128))

            wq_sb = bslice("wq", CO["wq"], CO["w2"])
            xtb_r = xtb_e.ap()              # [128, n_nc, 4, NCHUNK]
            xt_tiles = {}
            xt0 = xtp.tile([128, 4, NCHUNK], bf16, tag="xt")
            nc.sync.dma_start(out=xt0[:], in_=xtb_r[:, 0])
            xt_tiles[0] = xt0
            w2h = 16 * DIM // 2
            w2_sb = bnd.tile([128, 16, DIM], bf16, tag="b_w2")
            nc.sync.dma_start(
                out=w2_sb[:, 0:8].rearrange("p a b -> p (a b)"),
                in_=bund_e.ap()[:, CO["w2"]:CO["w2"] + w2h])
            xp_sb = bnd.tile([128, 16, m_pad], bf16, tag="b_xp")
            xph = 8 * m_pad
            nc.sync.dma_start(
                out=xp_sb[:, 0:8].rearrange("p a b -> p (a b)"),
                in_=bund_e.ap()[:, CO["xp"]:CO["xp"] + xph])
            nc.sync.dma_start(
                out=w2_sb[:, 8:16].rearrange("p a b -> p (a b)"),
                in_=bund_e.ap()[:, CO["w2"] + w2h:CO["xp"]])
            nc.sync.dma_start(
                out=xp_sb[:, 8:16].rearrange("p a b -> p (a b)"),
                in_=bund_e.ap()[:, CO["xp"] + xph:CO["wk"]])
            wk_sb = bslice("wk", CO["wk"], CO["wv"])
            wv_sb = bslice("wv", CO["wv"], CO["wp"])
            xt1 = xtp.tile([128, 4, NCHUNK], bf16, tag="xt")
            nc.sync.dma_start(out=xt1[:], in_=xtb_r[:, 1])
            xt_tiles[1] = xt1
            wp_sb = bslice("wp", CO["wp"], CO["end"])

            def w2kc(kc):
                return w2_sb[:, kc, :]

            def xpkc(kc, p0, p1):
                return xp_sb[:, kc, p0:p1]

            # ---- persistent PSUM scores tiles: 4-bank A + 1-bank D ----
            # (separate tiles so Tile's tile-granular dep tracking lets the
            # next head's mc4 score matmul bypass the big eA read)
            sA4 = s5pool.tile([128, nA, NCHUNK], f32, tag="sA4")
            # prologue-phase alias of the sDt bank: LN scratch
            lnp = s5pool.tile([128, NCHUNK], f32, tag="sDt")

            # ---- Q projection helpers ----
            def q_chain(xt_sb, q_sb, ic, psum_pool=None, psum_tag="q"):
                pool_ = psum_pool or qpp
                ps = pool_.tile([128, NCHUNK], f32, tag=psum_tag)
                for cc in range(4):
                    nc.tensor.matmul(
                        ps[:],
                        lhsT=r(wq_sb[:, cc, ic * 128:(ic + 1) * 128]),
                        rhs=r(xt_sb[:, cc, :]),
                        start=(cc == 0), stop=(cc == 3),
                    )
                nc.vector.tensor_copy(out=q_sb[:, ic, :], in_=ps[:])

            def q_swap_dma(q_sb, q_sw):
                # partition-shift copies via DMA (slow path on DVE)
                nc.gpsimd.dma_start(out=q_sw[0:64, :, :], in_=q_sb[64:128, :, :])
                nc.gpsimd.dma_start(out=q_sw[64:128, :, :], in_=q_sb[0:64, :, :])

            # ---- HAM warm-up: ~3.5us of tiny dependency-free matmuls so
            # Q(0) and the patch run at 2.4 GHz from the start ----
            warm_ps = pvp.tile([8, 64], f32, tag="pv")
            for _ in range(52):
                nc.tensor.matmul(warm_ps[0:8, 0:8], lhsT=r(ones8_sb[0:1, :]),
                                 rhs=r(ones8_sb[0:1, :]),
                                 start=True, stop=True)

            # ---- prologue Q for chunk 0 (PE work during DMA wait);
            # alternate PSUM banks so the q-copy WAR doesn't serialize ----
            q_tiles = {}
            q_sb0 = qp.tile([128, 4, NCHUNK], bf16, tag="q")
            q_sw0 = qp.tile([128, 4, NCHUNK], bf16, tag="qsw")
            q_tiles[0] = (q_sb0, q_sw0)
            for ic in range(4):
                q_chain(xt_tiles[0], q_tiles[0][0], ic,
                        psum_pool=(qpp if ic % 2 == 0 else fpp),
                        psum_tag=("q" if ic % 2 == 0 else "fin"))
            q_swap_dma(*q_tiles[0])

            # ---- phase 1: patch-merge ctx^T (feature-major) + layernorm ----
            ctx_raw = ctxp.tile([128, 4, m_pad], bf16, tag="craw")
            ctxn = ctxp.tile([128, 4, m_pad], bf16, tag="cn")
            k_sb = kvp.tile([128, 4, m_pad], bf16, tag="k")
            k_sw = kvp.tile([128, 4, m_pad], bf16, tag="ksw")
            v2_sb = kvp.tile([128, nmc, HEADS * EH], bf16, tag="v2")
            bv3 = bv_bc[:].rearrange("p (h d) -> p h d", d=DH)

            mu_sb = smp.tile([1, m_pad], f32, tag="mus")
            ss_sb = smp.tile([1, m_pad], f32, tag="sss")
            rowr = smp.tile([1, m_pad], f32, tag="rowr")
            rowt = smp.tile([1, m_pad], f32, tag="rowt")
            rt = smp.tile([128, 2 * nmc], f32, tag="rt")

            def patch_mms(p0, p1, kcs, ki0):
                """Patch-merge matmuls for m-range [p0,p1), kc subset."""
                pw = p1 - p0
                for ki, kc in enumerate(kcs):
                    for cco in range(4):
                        nc.tensor.matmul(
                            sA4[:, cco, :pw],
                            lhsT=r(w2kc(kc)[:, cco * 128:(cco + 1) * 128]),
                            rhs=r(xpkc(kc, p0, p1)),
                            start=(ki + ki0 == 0), stop=(ki + ki0 == 15),
                        )

            def stats_rows(p0, p1):
                """bias add + squares + column-stat matmul rows for a piece."""
                pw = p1 - p0
                for cco in range(4):
                    nc.vector.tensor_scalar(
                        out=ctx_raw[:, cco, p0:p1], in0=sA4[:, cco, :pw],
                        scalar1=convb_sb[:, cco:cco + 1], scalar2=None,
                        op0=OP.add,
                    )
                mu_ps = qpp.tile([1, NCHUNK], f32, tag="q")
                ss_ps = fpp.tile([1, NCHUNK], f32, tag="fin")
                for cc in range(4):
                    sq_s = sqp.tile([128, NCHUNK], bf16, tag="sqs")
                    nc.vector.tensor_tensor(
                        out=sq_s[:, :pw], in0=ctx_raw[:, cc, p0:p1],
                        in1=ctx_raw[:, cc, p0:p1], op=OP.mult,
                    )
                    nc.tensor.matmul(
                        mu_ps[0:1, :pw], lhsT=r(ones_sb[:]),
                        rhs=r(ctx_raw[:, cc, p0:p1]),
                        start=(cc == 0), stop=(cc == 3),
                    )
                    nc.tensor.matmul(
                        ss_ps[0:1, :pw], lhsT=r(ones_sb[:]),
                        rhs=r(sq_s[:, :pw]),
                        start=(cc == 0), stop=(cc == 3),
                    )
                nc.vector.tensor_copy(out=mu_sb[0:1, p0:p1],
                                      in_=mu_ps[0:1, :pw])
                nc.vector.tensor_copy(out=ss_sb[0:1, p0:p1],
                                      in_=ss_ps[0:1, :pw])

            def ln_chunk(j):
                """Scalar LN chain for m-chunk j in transposed [128,1] form."""
                c0 = 128 * j
                # transpose stat rows [1,128] -> [128,1] (cols of lnp)
                tin = lnp[:, 8 * j:8 * j + 2]
                nc.tensor.transpose(out=tin[:, 0:1],
                                    in_=mu_sb[0:1, c0:c0 + 128],
                                    identity=id2[0:1, 0:1])
                nc.tensor.transpose(out=tin[:, 1:2],
                                    in_=ss_sb[0:1, c0:c0 + 128],
                                    identity=id2[0:1, 0:1])
                m1n = smp.tile([128, 1], f32, tag="m1n")
                nc.vector.tensor_scalar(
                    out=m1n[:], in0=tin[:, 0:1],
                    scalar1=-1.0 / DIM, scalar2=None, op0=OP.mult)
                v1 = smp.tile([128, 1], f32, tag="v1")
                nc.vector.tensor_scalar(
                    out=v1[:], in0=tin[:, 1:2],
                    scalar1=1.0 / DIM, scalar2=None, op0=OP.mult)
                m2 = smp.tile([128, 1], f32, tag="m2")
                nc.vector.tensor_tensor(
                    out=m2[:], in0=m1n[:], in1=m1n[:], op=OP.mult)
                var = smp.tile([128, 1], f32, tag="var")
                nc.vector.tensor_tensor(
                    out=var[:], in0=v1[:], in1=m2[:], op=OP.subtract)
                std = smp.tile([128, 1], f32, tag="std")
                nc.scalar.activation(
                    out=std[:], in_=var[:], func=FT.Sqrt, bias=eps128[:])
                nc.vector.reciprocal(out=rt[:, 2 * j:2 * j + 1], in_=std[:])
                nc.vector.tensor_tensor(
                    out=rt[:, 2 * j + 1:2 * j + 2], in0=m1n[:],
                    in1=rt[:, 2 * j:2 * j + 1], op=OP.mult)
                # transpose back [128,1] -> [1,128] (two separate outputs so
                # both land at partition 0)
                tbr = lnp[0:1, 128:256]
                tbt = lnp[0:1, 256:384]
                nc.tensor.transpose(out=tbr, in_=rt[:, 2 * j:2 * j + 1],
                                    identity=id128)
                nc.tensor.transpose(out=tbt, in_=rt[:, 2 * j + 1:2 * j + 2],
                                    identity=id128)
                nc.vector.tensor_copy(out=rowr[:, c0:c0 + 128], in_=tbr)
                nc.vector.tensor_copy(out=rowt[:, c0:c0 + 128], in_=tbt)

            def ln_apply(p0, p1):
                """Broadcast rstd/shift rows and normalize the piece."""
                pw = p1 - p0
                r_bc = bcp.tile([128, NCHUNK], f32, tag="rbc")
                nc.gpsimd.partition_broadcast(
                    out_ap=r_bc[:, :pw], in_ap=rowr[0:1, p0:p1])
                t_bc = bcp.tile([128, NCHUNK], f32, tag="tbc")
                nc.gpsimd.partition_broadcast(
                    out_ap=t_bc[:, :pw], in_ap=rowt[0:1, p0:p1])
                for cc in range(4):
                    nc.vector.tensor_tensor(
                        out=ctxn[:, cc, p0:p1], in0=ctx_raw[:, cc, p0:p1],
                        in1=r_bc[:, :pw], op=OP.mult,
                    )
                    nc.vector.tensor_tensor(
                        out=ctxn[:, cc, p0:p1], in0=ctxn[:, cc, p0:p1],
                        in1=t_bc[:, :pw], op=OP.add,
                    )

            def k_piece(p0, p1):
                pw = p1 - p0
                for kc in range(4):
                    ps = qpp.tile([128, NCHUNK], f32, tag="q")
                    for cc in range(4):
                        nc.tensor.matmul(
                            ps[:, :pw],
                            lhsT=r(wk_sb[:, cc, kc * 128:(kc + 1) * 128]),
                            rhs=r(ctxn[:, cc, p0:p1]),
                            start=(cc == 0), stop=(cc == 3),
                        )
                    nc.vector.tensor_scalar(
                        out=k_sb[:, kc, p0:p1], in0=ps[:, :pw],
                        scalar1=bk_sb[:, kc:kc + 1], scalar2=None, op0=OP.add,
                    )
                nc.vector.tensor_copy(
                    out=k_sw[0:64, :, p0:p1], in_=k_sb[64:128, :, p0:p1])
                nc.vector.tensor_copy(
                    out=k_sw[64:128, :, p0:p1], in_=k_sb[0:64, :, p0:p1])

            def v_piece(p0, p1):
                for mc in range(p0 // 128, p1 // 128):
                    ps = fpp.tile([128, NCHUNK], f32, tag="fin")
                    for cc in range(4):
                        nc.tensor.matmul(
                            ps[:],
                            lhsT=r(ctxn[:, cc, mc * 128:(mc + 1) * 128]),
                            rhs=r(wv_sb[:, cc, :]),
                            start=(cc == 0), stop=(cc == 3),
                        )
                    v3 = v2_sb[:, mc, :].rearrange("p (h e) -> p h e", e=EH)
                    nc.vector.tensor_tensor(
                        out=v3[:, :, 0:DH],
                        in0=ps[:].rearrange("p (h d) -> p h d", d=DH),
                        in1=bv3, op=OP.add,
                    )
                    nc.vector.tensor_scalar(
                        out=v3[:, :, 0:DH], in0=v3[:, :, 0:DH],
                        scalar1=maskc_sb[:, mc:mc + 1], scalar2=None,
                        op0=OP.mult,
                    )
                    nc.vector.tensor_scalar(
                        out=v3[:, :, DH:EH],
                        in0=ones8_sb[:].rearrange("p (h u) -> p h u", u=1),
                        scalar1=maskc_sb[:, mc:mc + 1], scalar2=None,
                        op0=OP.mult,
                    )

            (pa, pb) = pieces
            # patch A interleaved with xp slice arrival
            patch_mms(pa[0], pa[1], list(range(8)), 0)
            patch_mms(pa[0], pa[1], list(range(8, 16)), 8)
            stats_rows(*pa)
            for j in range(pa[0] // 128, pa[1] // 128):
                ln_chunk(j)
            ln_apply(*pa)
            patch_mms(pb[0], pb[1], list(range(8)), 0)
            patch_mms(pb[0], pb[1], list(range(8, 16)), 8)
            stats_rows(*pb)
            for j in range(pb[0] // 128, pb[1] // 128):
                ln_chunk(j)
            # re-warm the exp table set (Sqrt above switched sets) while the
            # PE chews on K/V
            warm2_sb = cpool.tile([1, 1], f32, tag="warm2")
            nc.scalar.activation(out=warm2_sb[:], in_=eps_sb[:], func=FT.Exp)
            ln_apply(*pb)
            k_piece(*pa)
            v_piece(*pa)
            k_piece(*pb)
            v_piece(*pb)

            # main-loop instance of the sDt bank (rotates the LN scratch)
            sDt = s5pool.tile([128, NCHUNK], f32, tag="sDt")

            # ---- phase 3: main n-chunk loop, head-slot pipeline ----
            def scores_mm(q_sb, q_sw, h, mc):
                hc = h // 2
                half = mc % 2
                if (h % 2) == half:
                    ksrc, qsrc = k_sb, q_sb
                else:
                    ksrc, qsrc = k_sw, q_sw
                hp = half * 64
                dst = sA4[:, mc, :] if mc < nA else sDt[:]
                nc.tensor.matmul(
                    dst,
                    lhsT=r(ksrc[hp:hp + 64, hc, mc * 128:(mc + 1) * 128]),
                    rhs=r(qsrc[hp:hp + 64, hc, :]),
                    start=True, stop=True,
                )

            def pv_drain(h, eA, eD, o_st):
                pv = pvp.tile([EH, NCHUNK], f32, tag="pv")
                for mc in range(nmc):
                    src = eA[:, mc, :] if mc < nA else eD[:]
                    nc.tensor.matmul(
                        pv[:],
                        lhsT=r(v2_sb[:, mc, h * EH:(h + 1) * EH]),
                        rhs=r(src),
                        start=(mc == 0), stop=(mc == nmc - 1),
                    )
                nc.vector.tensor_copy(out=o_st[:, h, :], in_=pv[:])

            def div_stage1(g, o_st):
                dT = r1p.tile([128, 2 * NCHUNK // 128], bf16, tag="dT")
                nc.sync.dma_start(
                    out=dT[:],
                    in_=o_st[DH:EH, 2 * g:2 * g + 2, :].rearrange(
                        "p a b -> p (a b)"))
                return dT

            def div_stage2(g, dT):
                rT = r1p.tile([128, 2 * NCHUNK // 128], bf16, tag="rT")
                with nc.allow_low_precision("bf16 softmax denoms"):
                    nc.vector.reciprocal(out=rT[:], in_=dT[:])
                rfl = r1p.tile([1, 2, NCHUNK], bf16, tag="rf")
                nc.sync.dma_start(
                    out=rfl[:].rearrange("p a b -> p (a b)"),
                    in_=rT[:])
                rbs = []
                for j in (0, 1):
                    rb = rbp.tile([64, NCHUNK], bf16, tag="rb")
                    nc.gpsimd.partition_broadcast(
                        out_ap=rb[:], in_ap=rfl[0:1, j, :])
                    rbs.append(rb)
                return rbs

            def div_stage3(g, rbs, o_st, o_sb):
                for j, hh in enumerate((2 * g, 2 * g + 1)):
                    nc.vector.tensor_tensor(
                        out=o_sb[(hh % 2) * 64:(hh % 2) * 64 + 64,
                                 hh // 2, :],
                        in0=o_st[0:DH, hh, :], in1=rbs[j],
                        op=OP.mult,
                    )

            def out_chain(o_sb, cc, n0):
                ps = fpp.tile([128, NCHUNK], f32, tag="fin")
                for ic in range(4):
                    nc.tensor.matmul(
                        ps[:],
                        lhsT=r(wp_sb[:, ic, cc * 128:(cc + 1) * 128]),
                        rhs=r(o_sb[:, ic, :]),
                        start=(ic == 0), stop=(ic == 3),
                    )
                y_sb = yp.tile([128, NCHUNK], f32, tag="y")
                nc.vector.tensor_scalar(
                    out=y_sb[:], in0=ps[:], scalar1=bp_sb[:, cc:cc + 1],
                    scalar2=None, op0=OP.add,
                )
                nc.sync.dma_start(
                    out=out_e.ap()[cc * 128:(cc + 1) * 128, n0:n0 + NCHUNK],
                    in_=y_sb[:],
                )

            prev = None       # (h, eA, eD, o_st, o_sb)
            prev_chunk = None  # (o_sb, n0) of previous chunk, for out-proj
            pending = []      # [(due_slot, fn)] deferred divide stages
            slot = 0

            def flush(s):
                ready = [p for p in pending if p[0] <= s]
                pending[:] = [p for p in pending if p[0] > s]
                for _, fn in ready:
                    fn()
            for ni in range(n_nc):
                n0 = ni * NCHUNK
                q_sb, q_sw = q_tiles[ni]
                o_sb = opool.tile([128, 4, NCHUNK], bf16, tag="o")
                o_st = opool.tile([EH, 8, NCHUNK], bf16, tag="ost")
                if 0 <= ni <= 6:
                    qn_sb = qp.tile([128, 4, NCHUNK], bf16, tag="q")
                    qn_sw = qp.tile([128, 4, NCHUNK], bf16, tag="qsw")
                    q_tiles[ni + 1] = (qn_sb, qn_sw)
                for h in range(HEADS):
                    flush(slot)
                    # scores mc 0..3 -> banks 0..3 (wait previous eA read)
                    for mc in range(nA):
                        scores_mm(q_sb, q_sw, h, mc)
                    eA = ep.tile([128, nA, NCHUNK], bf16, tag="eA")
                    eD = ep.tile([128, NCHUNK], bf16, tag="eD")
                    nc.scalar.activation(
                        out=eA[:], in_=sA4[:], func=FT.Exp)
                    # trailing PV for the previous head-slot
                    if prev is not None:
                        ph, peA, peD, po_st, po_sb = prev
                        pv_drain(ph, peA, peD, po_st)
                    # scores mc 4 -> bank 4 (waits previous eD read only)
                    scores_mm(q_sb, q_sw, h, nA)
                    nc.scalar.activation(
                        out=eD[:], in_=sDt[:], func=FT.Exp)
                    # gap-filler chains: Q-proj (next chunk) in slots 0-3,
                    # out-proj (previous chunk) in slots 4-7
                    if h < 4:
                        if 0 <= ni <= 6:
                            q_chain(xt_tiles[ni + 1], q_tiles[ni + 1][0], h)
                            if h == 3:
                                q_swap_dma(*q_tiles[ni + 1])
                        if h == 1 and 0 <= ni <= 5:
                            xt = xtp.tile([128, 4, NCHUNK], bf16, tag="xt")
                            nc.sync.dma_start(
                                out=xt[:], in_=xtb_r[:, ni + 2])
                            xt_tiles[ni + 2] = xt
                    else:
                        if prev_chunk is not None:
                            out_chain(prev_chunk[0], h - 4, prev_chunk[1])
                    # divides trail everything (sync-DMA transposes + gpsimd
                    # broadcasts + DVE), pipelined over the next slots
                    if prev is not None and prev[0] % 2 == 1:
                        g, d_ost, d_osb = prev[0] // 2, prev[3], prev[4]
                        dT = div_stage1(g, d_ost)

                        def mk2(g=g, dT=dT, d_ost=d_ost, d_osb=d_osb, s=slot):
                            rbs = div_stage2(g, dT)
                            pending.append(
                                (s + 2,
                                 lambda: div_stage3(g, rbs, d_ost, d_osb)))
                        pending.append((slot + 1, mk2))
                    prev = (h, eA, eD, o_st, o_sb)
                    slot += 1
                prev_chunk = (o_sb, n0)

            # ---- tail: last head's PV + divide overlap the final out-proj
            # (per-ic accumulation into the freed score PSUM banks) ----
            ph, peA, peD, po_st, po_sb = prev
            pv_drain(ph, peA, peD, po_st)
            dT3 = div_stage1(3, po_st)
            while pending:
                flush(slot + 10)
                slot += 10
            rbs3 = div_stage2(3, dT3)
            ps4 = s5pool.tile([128, nA, NCHUNK], f32, tag="sA4")
            for ic in range(3):
                for cc in range(4):
                    nc.tensor.matmul(
                        ps4[:, cc, :],
                        lhsT=r(wp_sb[:, ic, cc * 128:(cc + 1) * 128]),
                        rhs=r(po_sb[:, ic, :]),
                        start=(ic == 0), stop=False,
                    )
            div_stage3(3, rbs3, po_st, po_sb)
            n0 = prev_chunk[1]
            for cc in range(4):
                nc.tensor.matmul(
                    ps4[:, cc, :],
                    lhsT=r(wp_sb[:, 3, cc * 128:(cc + 1) * 128]),
                    rhs=r(po_sb[:, 3, :]),
                    start=False, stop=True,
                )
                y_sb = yp.tile([128, NCHUNK], f32, tag="y")
                nc.vector.tensor_scalar(
                    out=y_sb[:], in0=ps4[:, cc, :],
                    scalar1=bp_sb[:, cc:cc + 1], scalar2=None, op0=OP.add,
                )
                nc.sync.dma_start(
                    out=out_e.ap()[cc * 128:(cc + 1) * 128, n0:n0 + NCHUNK],
                    in_=y_sb[:],
                )

    nc.finalize()
    return nc


def _prep_inputs(x, mask, Wq, Wkv, conv_w, conv_b, ln_w, ln_b, Wp, bp, W):
    """Host-side sharding + layout prep. Returns (in_maps, m_pad)."""
    import ml_dtypes
    bf16 = ml_dtypes.bfloat16
    x = np.ascontiguousarray(np.asarray(x, dtype=np.float32))
    mask = np.asarray(mask, dtype=np.float32)
    Wq = np.asarray(Wq, dtype=np.float32)
    Wkv = np.asarray(Wkv, dtype=np.float32)
    conv_w = np.asarray(conv_w, dtype=np.float32)
    conv_b = np.asarray(conv_b, dtype=np.float32)
    ln_w = np.asarray(ln_w, dtype=np.float32)
    ln_b = np.asarray(ln_b, dtype=np.float32)
    Wp = np.asarray(Wp, dtype=np.float32)
    bp = np.asarray(bp, dtype=np.float32)

    Wm = W // SR
    kb = [int((mask[b] != 0).sum()) for b in range(B)]
    m_pad = max(256, ((max(kb) + 127) // 128) * 128)
    CO = _bundle_cols(m_pad)

    def rearr_w(w):  # [512, 512] -> [128, 4, 512] with [p, cc, :] = w[cc*128+p]
        return np.ascontiguousarray(w.reshape(4, 128, -1).transpose(1, 0, 2))

    def rearr_b(v):  # [512] -> [128, 4]
        return np.ascontiguousarray(v.reshape(4, 128).T)

    w2 = np.ascontiguousarray(
        conv_w.transpose(2, 3, 1, 0).reshape(4 * DIM, DIM))
    wq_in = rearr_w(Wq.T * np.float32(SCALE))
    wk_in = rearr_w((Wkv[:INNER] * ln_w).T)
    wv_in = rearr_w((Wkv[INNER:] * ln_w).T)
    wp_in = rearr_w(Wp.T)
    bk_in = rearr_b(Wkv[:INNER] @ ln_b)
    bv_in = np.ascontiguousarray(Wkv[INNER:] @ ln_b)
    convb_in = rearr_b(conv_b)
    bp_in = rearr_b(bp)

    in_maps = []
    for b in range(B):
        xb = x[b]
        xbT = np.ascontiguousarray(xb.T)  # [512, 4096]
        sel = np.nonzero(mask[b] != 0)[0]
        sel_pad = np.zeros(m_pad, dtype=np.int64)
        sel_pad[: len(sel)] = sel
        i = sel_pad // Wm
        j = sel_pad % Wm
        n_idx = np.stack(
            [(2 * i + di) * W + (2 * j + dj) for di in (0, 1) for dj in (0, 1)]
        )  # [4, m_pad], p = di*2+dj
        xp = xb[n_idx]  # [4, m_pad, 512]
        xp = np.ascontiguousarray(
            xp.transpose(0, 2, 1).reshape(4 * DIM, m_pad))
        maskc = (np.arange(m_pad) < len(sel)).astype(np.float32)
        maskc_in = np.ascontiguousarray(maskc.reshape(-1, 128).T)

        # ---- bf16 bundle [128, C]: wq | w2 | xp | wk | wv | wp ----
        bund = np.zeros((128, CO["end"]), dtype=np.float32)
        bund[:, CO["wq"]:CO["w2"]] = wq_in.reshape(128, -1)
        # w2 slice [p, kc, co] = w2[kc*128+p, co]
        bund[:, CO["w2"]:CO["xp"]] = np.ascontiguousarray(
            w2.reshape(16, 128, DIM).transpose(1, 0, 2)).reshape(128, -1)
        # xp slice [p, kc, m] = xp[kc*128+p, m]
        bund[:, CO["xp"]:CO["wk"]] = np.ascontiguousarray(
            xp.reshape(16, 128, m_pad).transpose(1, 0, 2)).reshape(128, -1)
        bund[:, CO["wk"]:CO["wv"]] = wk_in.reshape(128, -1)
        bund[:, CO["wv"]:CO["wp"]] = wv_in.reshape(128, -1)
        bund[:, CO["wp"]:CO["end"]] = wp_in.reshape(128, -1)

        # xt chunks, contiguous per chunk: [p, k, cc, n] = xT[cc*128+p, 512k+n]
        xtb = np.ascontiguousarray(
            xbT.reshape(4, 128, N_SEQ // NCHUNK, NCHUNK)
            .transpose(1, 2, 0, 3))

        cst = np.zeros((128, 145), dtype=np.float32)
        cst[:, 0:4] = convb_in
        cst[:, 4:8] = bk_in
        cst[:, 8:12] = bp_in
        cst[:, 12:17] = maskc_in
        cst[:, 17:145] = np.eye(128, dtype=np.float32)
        in_maps.append({
            "bund": bund.astype(bf16),
            "xtb": xtb.astype(bf16),
            "cst": cst,
            "bv": bv_in,
        })
    return in_maps, m_pad


_BUILD_CACHE = {}


def kernel(x, H, W, mask, Wq, Wkv, conv_w, conv_b, ln_w, ln_b, Wp, bp,
           _results_hook=None):
    H = int(H)
    W = int(W)
    assert (H, W) == (64, 64) and x.shape == (B, N_SEQ, DIM), (H, W, x.shape)

    in_maps, m_pad = _prep_inputs(
        x, mask, Wq, Wkv, conv_w, conv_b, ln_w, ln_b, Wp, bp, W)

    if m_pad not in _BUILD_CACHE:
        _BUILD_CACHE[m_pad] = _build(m_pad)
    nc = _BUILD_CACHE[m_pad]

    _ensure_path()
    from concourse.bass_utils import run_bass_kernel_spmd

    res = run_bass_kernel_spmd(nc, in_maps, core_ids=list(range(B)))
    if _results_hook is not None:
        _results_hook(res)

    out = np.empty((B, N_SEQ, DIM), dtype=np.float32)
    for b in range(B):
        out[b] = res.results[b]["out"].T
    return out
